# revision 1
# baseline (speedup 1.0000x reference)
"""Single-head attention (B=4, S=2048, E=1024) on 8 TRN2 NeuronCores.

Sharding: data-parallel over (batch, query-half): core c handles batch c//2,
queries [h*1024, (h+1)*1024) where h = c%2. Each core computes K/V for its
full batch (duplicated within the pair) so there are no collectives.

Per-core layout trick: the host permutes the key/value sequence so that this
core's query block is always columns [0, 1024) of xT. Attention output is
invariant to a consistent permutation of keys+values, so K/V built from the
permuted xT give identical results.

Pipeline (all matmuls bf16 inputs, fp32 PSUM accumulation):
  KT[f,s]  = WkT.T @ xT          (bk dropped: constant-per-query shift,
                                  softmax-invariant)
  QT[f,sq] = WqT.T @ xT[:, :1024] + bq
  V[s,e]   = xT.T @ WvT
  scores[sq,sk] = QT.T @ KT  (per 128-row q-block, 512-wide sk groups)
  attn = exp(scores/32)           (unnormalized; scores ~ N(0,1), no
                                  max-subtraction needed; row-sums via
                                  activation accum_out)
  out[sq,e] = (attnT.T @ V) * (1/rowsum) + bv
"""
import numpy as np
import ml_dtypes

import concourse.bass as bass
import concourse.bacc as bacc
import concourse.mybir as mybir
from concourse.tile import TileContext
from concourse.bass_utils import run_bass_kernel_spmd
from concourse.masks import make_identity

B, S, E = 4, 2048, 1024
P = 128
EC = E // P          # 8 contraction chunks
FC = E // P          # 8 feature chunks
SKC = S // P         # 16 key chunks
SQ = S // 2          # queries per core
QB = SQ // P         # 8 query blocks per core
NG = 512             # moving-dim tile
INV_SCALE = 1.0 / float(np.sqrt(E))

MM_DT = mybir.dt.bfloat16
NP_MM = ml_dtypes.bfloat16
F32 = mybir.dt.float32

_CACHE = {}


def _build():
    nc = bacc.Bacc()
    xt = nc.declare_dram_parameter("xt", [E, S], MM_DT, isOutput=False)
    wqc = nc.declare_dram_parameter("wqc", [FC, P, EC, P], MM_DT, isOutput=False)
    wkc = nc.declare_dram_parameter("wkc", [FC, P, EC, P], MM_DT, isOutput=False)
    wvc = nc.declare_dram_parameter("wvc", [E // NG, P, EC, NG], MM_DT, isOutput=False)
    bqr = nc.declare_dram_parameter("bqr", [P, FC], F32, isOutput=False)
    bvb = nc.declare_dram_parameter("bvb", [P, E], F32, isOutput=False)
    out = nc.declare_dram_parameter("out", [SQ, E], F32, isOutput=True)

    xt_r = xt[:, :].rearrange("(ec p) s -> p ec s", p=P)

    with TileContext(nc) as tc:
        with (
            tc.tile_pool(name="wp", bufs=1) as wp,
            tc.tile_pool(name="kvq", bufs=1) as kvq,
            tc.tile_pool(name="att", bufs=2) as att_pool,
            tc.tile_pool(name="attT", bufs=2) as attT_pool,
            tc.tile_pool(name="outp", bufs=2) as outp,
            tc.tile_pool(name="smalls", bufs=2) as smalls,
            tc.tile_pool(name="ps", bufs=3, space="PSUM") as ps,
            tc.tile_pool(name="pstr", bufs=3, space="PSUM") as pstr,
        ):
            # ---- loads (ordered so the K projection can start earliest) ----
            ident = wp.tile([P, P], MM_DT)
            make_identity(nc, ident)

            w_sb = {}
            for name in ("wq", "wk", "wv"):
                w_sb[name] = wp.tile([P, EC, E], MM_DT, name=f"{name}_sb")
            xt_sb = wp.tile([P, EC, S], MM_DT)

            def load_xt_group(g):
                for ec in range(EC):
                    nc.sync.dma_start(
                        xt_sb[:, ec, g * NG:(g + 1) * NG],
                        xt_r[:, ec, g * NG:(g + 1) * NG],
                    )

            nc.sync.dma_start(w_sb["wk"][:, :, 0:P], wkc[0])
            load_xt_group(0)
            for fc in range(1, FC):
                nc.sync.dma_start(w_sb["wk"][:, :, fc * P:(fc + 1) * P], wkc[fc])
            load_xt_group(1)
            load_xt_group(2)
            load_xt_group(3)
            for fc in range(FC):
                nc.sync.dma_start(w_sb["wq"][:, :, fc * P:(fc + 1) * P], wqc[fc])
            for g in range(E // NG):
                nc.sync.dma_start(w_sb["wv"][:, :, g * NG:(g + 1) * NG], wvc[g])
            bq_sb = wp.tile([P, FC], F32)
            nc.sync.dma_start(bq_sb[:], bqr[:, :])
            bv_sb = wp.tile([P, E], F32)
            nc.sync.dma_start(bv_sb[:], bvb[:, :])

            # PE warmup: cover the initial DMA latency and release the HAM
            # clock throttle before real matmuls arrive (~8us of transposes,
            # serialized by WAW on one PSUM tile; results unused).
            warm_ps = ps.tile([P, P], MM_DT, tag="pv", bufs=2)
            for _ in range(60):
                nc.tensor.transpose(warm_ps[:], ident[:], ident[:])

            KT = kvq.tile([P, FC, S], MM_DT)
            QT = kvq.tile([P, FC, SQ], MM_DT)
            V = kvq.tile([P, SKC, E], MM_DT)

            # ---- K projection (KT[f, sk]), g-major to match DMA stream ----
            for g in range(S // NG):
                for fc in range(FC):
                    pk = ps.tile([P, NG], F32, tag="mm")
                    for ec in range(EC):
                        nc.tensor.matmul(
                            pk[:],
                            w_sb["wk"][:, ec, fc * P:(fc + 1) * P],
                            xt_sb[:, ec, g * NG:(g + 1) * NG],
                            start=(ec == 0),
                            stop=(ec == EC - 1),
                        )
                    nc.scalar.copy(KT[:, fc, g * NG:(g + 1) * NG], pk[:])

            # ---- Q projection (QT[f, sq] + bq) ----
            for fc in range(FC):
                for g in range(SQ // NG):
                    pq = ps.tile([P, NG], F32, tag="mm")
                    for ec in range(EC):
                        nc.tensor.matmul(
                            pq[:],
                            w_sb["wq"][:, ec, fc * P:(fc + 1) * P],
                            xt_sb[:, ec, g * NG:(g + 1) * NG],
                            start=(ec == 0),
                            stop=(ec == EC - 1),
                        )
                    nc.scalar.activation(
                        QT[:, fc, g * NG:(g + 1) * NG],
                        pq[:],
                        mybir.ActivationFunctionType.Identity,
                        bias=bq_sb[:, fc:fc + 1],
                    )

            # ---- V projection (V[sk, e]) ----
            for skc in range(SKC):
                for g in range(E // NG):
                    pv = ps.tile([P, NG], F32, tag="mm")
                    for ec in range(EC):
                        nc.tensor.matmul(
                            pv[:],
                            xt_sb[:, ec, skc * P:(skc + 1) * P],
                            w_sb["wv"][:, ec, g * NG:(g + 1) * NG],
                            start=(ec == 0),
                            stop=(ec == EC - 1),
                        )
                    nc.vector.tensor_copy(V[:, skc, g * NG:(g + 1) * NG], pv[:])

            # ---- attention per q-block ----
            for qb in range(QB):
                qsl = slice(qb * P, (qb + 1) * P)
                attn = att_pool.tile([P, S], MM_DT, tag="attn")
                sums4 = smalls.tile([P, S // NG], F32, tag="s4")
                for g in range(S // NG):
                    pscr = ps.tile([P, NG], F32, tag="mm")
                    for fc in range(FC):
                        nc.tensor.matmul(
                            pscr[:],
                            QT[:, fc, qsl],
                            KT[:, fc, g * NG:(g + 1) * NG],
                            start=(fc == 0),
                            stop=(fc == FC - 1),
                        )
                    nc.scalar.activation(
                        attn[:, g * NG:(g + 1) * NG],
                        pscr[:],
                        mybir.ActivationFunctionType.Exp,
                        scale=float(INV_SCALE),
                        accum_out=sums4[:, g:g + 1],
                    )
                ssum = smalls.tile([P, 1], F32, tag="ssum")
                nc.vector.reduce_sum(ssum[:], sums4[:], axis=mybir.AxisListType.X)
                recip = smalls.tile([P, 1], F32, tag="recip")
                nc.vector.reciprocal(recip[:], ssum[:])

                attT = attT_pool.tile([P, SKC, P], MM_DT, tag="attT")
                for skc in range(SKC):
                    pt = pstr.tile([P, P], MM_DT, tag="tr")
                    nc.tensor.transpose(pt[:], attn[:, skc * P:(skc + 1) * P], ident[:])
                    nc.vector.tensor_copy(attT[:, skc], pt[:])

                outt = outp.tile([P, E], F32, tag="out")
                for g in range(E // NG):
                    ppv = ps.tile([P, NG], F32, tag="pv", bufs=2)
                    for skc in range(SKC):
                        nc.tensor.matmul(
                            ppv[:],
                            attT[:, skc],
                            V[:, skc, g * NG:(g + 1) * NG],
                            start=(skc == 0),
                            stop=(skc == SKC - 1),
                        )
                    nc.scalar.activation(
                        outt[:, g * NG:(g + 1) * NG],
                        ppv[:],
                        mybir.ActivationFunctionType.Copy,
                        scale=recip[:, 0:1],
                    )
                    nc.vector.tensor_add(
                        outt[:, g * NG:(g + 1) * NG],
                        outt[:, g * NG:(g + 1) * NG],
                        bv_sb[:, g * NG:(g + 1) * NG],
                    )
                    nc.sync.dma_start(
                        out[qb * P:(qb + 1) * P, g * NG:(g + 1) * NG],
                        outt[:, g * NG:(g + 1) * NG],
                    )
    nc.finalize()
    return nc


def build_in_maps(x, Wq, bq, Wk, bk, Wv, bv):
    x = np.asarray(x, dtype=np.float32)

    def colchunk(W, n):
        # W.T is [E(e), E(f)]; -> [E//n(fchunk), P(p of e), EC(ec), n]
        wt = np.ascontiguousarray(np.asarray(W, np.float32).T).astype(NP_MM)
        return np.ascontiguousarray(
            wt.reshape(EC, P, E // n, n).transpose(2, 1, 0, 3)
        )

    wqc = colchunk(Wq, P)
    wkc = colchunk(Wk, P)
    wvc = colchunk(Wv, NG)
    bqr = np.ascontiguousarray(
        np.asarray(bq, np.float32).reshape(FC, P).T
    )  # [P, FC]; column fc = bq[fc*128:(fc+1)*128]
    bvb = np.broadcast_to(np.asarray(bv, np.float32)[None, :], (P, E)).copy()

    in_maps = []
    for c in range(8):
        b, h = divmod(c, 2)
        xt_full = np.ascontiguousarray(x[b].T).astype(NP_MM)  # [E, S]
        if h == 0:
            xt_perm = xt_full
        else:
            xt_perm = np.ascontiguousarray(
                np.concatenate([xt_full[:, SQ:], xt_full[:, :SQ]], axis=1)
            )
        in_maps.append(
            dict(xt=xt_perm, wqc=wqc, wkc=wkc, wvc=wvc, bqr=bqr, bvb=bvb)
        )

    return in_maps


def kernel(x, Wq, bq, Wk, bk, Wv, bv):
    if "nc" not in _CACHE:
        _CACHE["nc"] = _build()
    nc = _CACHE["nc"]
    in_maps = build_in_maps(x, Wq, bq, Wk, bk, Wv, bv)
    res = run_bass_kernel_spmd(nc, in_maps, list(range(8)))

    out = np.empty((B, S, E), np.float32)
    for c in range(8):
        b, h = divmod(c, 2)
        out[b, h * SQ:(h + 1) * SQ, :] = res.results[c]["out"]
    return out



# revision 2
# speedup vs baseline: 1.1100x; 1.1100x over previous
"""Single-head attention (B=4, S=2048, E=1024) on 8 TRN2 NeuronCores, v2.

Sharding: data-parallel over (batch, query-half) as v1: core c handles batch
c//2, queries [h*1024,(h+1)*1024), h=c%2. Host permutes the key sequence so
this core's query block is always positions [0,1024) (attention is invariant
to a consistent key permutation).

Algorithmic restructuring vs v1 (eliminates K/Q/V projections on device):
  scores = q.k = (x Wq^T + bq)(x Wk^T + bk)^T
         = x M x^T + [row-const terms] + t2[k],  M = Wq^T Wk (host, 1 GFLOP)
         t2[k] = x_k . (Wk^T bq)                 (host matvec, O(S*E))
  row-constant terms are softmax-invariant and dropped.
  out = attn @ v = attn @ (x Wv^T + bv) = (attn @ x) @ Wv^T + bv
  (rows of attn sum to 1 after normalization).

Device pipeline (bf16 matmuls, fp32 PSUM):
  xMT[e,q]   = M^T @ xT[:, :1024]            (128 mm)
  scoresT[k,q] chunks = xT_chunk^T @ xMT     (256 mm)
  attT = exp(scoresT/sqrt(E) + t2/sqrt(E))   (scalar engine, direct transposed
                                              layout - no PE transposes)
  sums[q] += attT_chunk^T @ ones             (128 N=1 mm)
  TT[f,q]    = x_chunk^T @ attT              (256 mm; T = attn @ x transposed)
  out[q,e]   = (TT_chunk^T @ WvT) * recip + bv   (128 mm)
"""
import numpy as np
import ml_dtypes

import concourse.bass as bass
import concourse.bacc as bacc
import concourse.mybir as mybir
from concourse.tile import TileContext
from concourse.bass_utils import run_bass_kernel_spmd

B, S, E = 4, 2048, 1024
P = 128
EC = E // P          # 8 feature chunks
SKC = S // P         # 16 key chunks
SQ = S // 2          # queries per core
QB = SQ // P         # 8 query blocks per core
NG = 512             # moving-dim tile
QG = SQ // NG        # 2 query groups
INV_SCALE = 1.0 / float(np.sqrt(E))

MM_DT = mybir.dt.bfloat16
NP_MM = ml_dtypes.bfloat16
F32 = mybir.dt.float32

_CACHE = {}


def _build():
    nc = bacc.Bacc()
    xtc = nc.declare_dram_parameter("xtc", [EC, P, S], MM_DT, isOutput=False)
    xsc = nc.declare_dram_parameter("xsc", [SKC, P, E], MM_DT, isOutput=False)
    mc = nc.declare_dram_parameter("mc", [EC, P, E], MM_DT, isOutput=False)
    wvc = nc.declare_dram_parameter("wvc", [EC, P, E], MM_DT, isOutput=False)
    t2b = nc.declare_dram_parameter("t2b", [P, SKC], F32, isOutput=False)
    bvb = nc.declare_dram_parameter("bvb", [P, E], F32, isOutput=False)
    onesb = nc.declare_dram_parameter("onesb", [P, 1], MM_DT, isOutput=False)
    out = nc.declare_dram_parameter("out", [SQ, E], F32, isOutput=True)

    with TileContext(nc) as tc:
        with (
            tc.tile_pool(name="wp", bufs=1) as wp,
            tc.tile_pool(name="big", bufs=1) as big,
            tc.tile_pool(name="outp", bufs=2) as outp,
            tc.tile_pool(name="smalls", bufs=2) as smalls,
            tc.tile_pool(name="ps", bufs=3, space="PSUM") as ps,
            tc.tile_pool(name="pss", bufs=1, space="PSUM") as pss,
        ):
            m_sb = wp.tile([P, EC, E], MM_DT)     # M slab ech: rows ec*128+j
            xt_sb = wp.tile([P, EC, S], MM_DT)    # xT: feat rows, seq cols
            xs_sb = wp.tile([P, SKC, E], MM_DT)   # x:  seq rows, feat cols
            wv_sb = wp.tile([P, EC, E], MM_DT)    # WvT: feat rows, e cols
            t2_sb = wp.tile([P, SKC], F32)
            bv_sb = wp.tile([P, E], F32)
            ones_sb = wp.tile([P, 1], MM_DT)
            warm_in = wp.tile([P, P], MM_DT)
            nc.vector.memset(warm_in[:], 1.0)

            # ---- DMA loads, sync queue in consumption order, consolidated
            # multi-chunk transfers (2KB/1KB lines, few dispatches).
            xtc_r = xtc[:, :, :].rearrange("ec p s -> p ec s")
            mc_r = mc[:, :, :].rearrange("c p e -> p c e")
            nc.sync.dma_start(m_sb[:, 0, :], mc[0])
            nc.sync.dma_start(xt_sb[:, 0:4, 0:NG], xtc_r[:, 0:4, 0:NG])
            nc.sync.dma_start(xt_sb[:, 4:8, 0:NG], xtc_r[:, 4:8, 0:NG])
            nc.sync.dma_start(m_sb[:, 1:4, :], mc_r[:, 1:4, :])
            nc.sync.dma_start(m_sb[:, 4:8, :], mc_r[:, 4:8, :])
            nc.sync.dma_start(xt_sb[:, :, NG:2 * NG], xtc_r[:, :, NG:2 * NG])
            nc.sync.dma_start(xt_sb[:, :, 2 * NG:3 * NG], xtc_r[:, :, 2 * NG:3 * NG])
            nc.sync.dma_start(xt_sb[:, :, 3 * NG:S], xtc_r[:, :, 3 * NG:S])
            nc.gpsimd.dma_start(ones_sb[:], onesb[:, :])
            nc.gpsimd.dma_start(t2_sb[:], t2b[:, :])

            # PE warmup: cover preamble/DMA latency, release HAM throttle.
            warm_ps = ps.tile([P, P], F32, tag="warm", bufs=2)
            for _ in range(64):
                nc.tensor.matmul(warm_ps[:], warm_in[:], warm_in[:],
                                 start=True, stop=True)

            xmt_sb = big.tile([P, EC, SQ], MM_DT)   # (xM)^T: e rows, q cols
            att_sb = big.tile([P, SKC, SQ], MM_DT)  # attT: k rows, q cols
            tt_sb = big.tile([P, EC, SQ], MM_DT)    # (attn@x)^T: f rows, q cols

            def emit_p1(qg):
                # xMT[e, q] = M^T @ xT (own queries)
                for ech in range(EC):
                    pq = ps.tile([P, NG], F32, tag="mm")
                    for ec in range(EC):
                        nc.tensor.matmul(
                            pq[:],
                            m_sb[:, ech, ec * P:(ec + 1) * P],
                            xt_sb[:, ec, qg * NG:(qg + 1) * NG],
                            start=(ec == 0),
                            stop=(ec == EC - 1),
                        )
                    nc.scalar.copy(
                        xmt_sb[:, ech, qg * NG:(qg + 1) * NG], pq[:]
                    )

            def emit_p2(qg):
                # scoresT chunks -> exp -> attT
                for skc in range(SKC):
                    pscr = ps.tile([P, NG], F32, tag="mm")
                    for ec in range(EC):
                        nc.tensor.matmul(
                            pscr[:],
                            xt_sb[:, ec, skc * P:(skc + 1) * P],
                            xmt_sb[:, ec, qg * NG:(qg + 1) * NG],
                            start=(ec == 0),
                            stop=(ec == EC - 1),
                        )
                    nc.scalar.activation(
                        att_sb[:, skc, qg * NG:(qg + 1) * NG],
                        pscr[:],
                        mybir.ActivationFunctionType.Exp,
                        scale=float(INV_SCALE),
                        bias=t2_sb[:, skc:skc + 1],
                    )

            emit_p1(0)
            emit_p2(0)
            emit_p1(1)

            # xs loads must not contend with the critical M/xT stream: give
            # each destination slab a WAR dependency on late phase-1 data so
            # the scheduler cannot hoist the transfers to kernel start.
            # (xs is first needed in phase 3, ~40us later.)
            gsync = smalls.tile([P, 1], MM_DT, tag="gsync")
            nc.gpsimd.tensor_copy(gsync[:], xmt_sb[:, EC - 1, SQ - 1:SQ])
            for skc in range(SKC):
                nc.gpsimd.tensor_copy(xs_sb[:, skc, 0:1], gsync[:])
                nc.gpsimd.dma_start(xs_sb[:, skc], xsc[skc])

            emit_p2(1)

            # wv/bv: same trick, anchored on the end of the qg0 exps
            # (phase 4 needs wv much later).
            wsync = smalls.tile([P, 1], MM_DT, tag="wsync")
            nc.scalar.copy(wsync[:], att_sb[:, SKC - 1, NG - 1:NG])
            for ec in range(EC):
                nc.scalar.copy(wv_sb[:, ec, 0:1], wsync[:])
                nc.scalar.dma_start(wv_sb[:, ec], wvc[ec])
            nc.scalar.copy(bv_sb[:, 0:1], wsync[:])
            nc.scalar.dma_start(bv_sb[:], bvb[:, :])

            # ---- Phase 3: TT[f, q] = x^T @ attT, rowsum chains interleaved
            psum_sums = pss.tile([P, QB], F32)

            def emit_sums(qb):
                for skc in range(SKC):
                    nc.tensor.matmul(
                        psum_sums[:, qb:qb + 1],
                        att_sb[:, skc, qb * P:(qb + 1) * P],
                        ones_sb[:, 0:1],
                        start=(skc == 0),
                        stop=(skc == SKC - 1),
                    )

            def emit_p3(qg):
                for ech in range(EC):
                    pt = ps.tile([P, NG], F32, tag="mm")
                    for skc in range(SKC):
                        nc.tensor.matmul(
                            pt[:],
                            xs_sb[:, skc, ech * P:(ech + 1) * P],
                            att_sb[:, skc, qg * NG:(qg + 1) * NG],
                            start=(skc == 0),
                            stop=(skc == SKC - 1),
                        )
                    nc.vector.tensor_copy(
                        tt_sb[:, ech, qg * NG:(qg + 1) * NG], pt[:]
                    )
                    if ech < 4:
                        emit_sums(qg * 4 + ech)

            def emit_recip(qg):
                # recip for this qg's q-blocks (partition-aligned)
                sums_sb = smalls.tile([P, 4], F32, tag=f"sums{qg}")
                nc.vector.tensor_copy(
                    sums_sb[:], psum_sums[:, qg * 4:(qg + 1) * 4]
                )
                recip = smalls.tile([P, 4], F32, tag=f"recip{qg}")
                nc.vector.reciprocal(recip[:], sums_sb[:])
                return recip

            def emit_p4(qg, recip):
                # out[q, e] = (TT^T @ WvT) * recip + bv for this qg's blocks
                for qbl in range(4):
                    qb = qg * 4 + qbl
                    outt = outp.tile([P, E], F32, tag="out")
                    for eg in range(E // NG):
                        po = ps.tile([P, NG], F32, tag="mm")
                        for ec in range(EC):
                            nc.tensor.matmul(
                                po[:],
                                tt_sb[:, ec, qb * P:(qb + 1) * P],
                                wv_sb[:, ec, eg * NG:(eg + 1) * NG],
                                start=(ec == 0),
                                stop=(ec == EC - 1),
                            )
                        nc.scalar.activation(
                            outt[:, eg * NG:(eg + 1) * NG],
                            po[:],
                            mybir.ActivationFunctionType.Copy,
                            scale=recip[:, qbl:qbl + 1],
                        )
                        nc.vector.tensor_add(
                            outt[:, eg * NG:(eg + 1) * NG],
                            outt[:, eg * NG:(eg + 1) * NG],
                            bv_sb[:, eg * NG:(eg + 1) * NG],
                        )
                        nc.sync.dma_start(
                            out[qb * P:(qb + 1) * P, eg * NG:(eg + 1) * NG],
                            outt[:, eg * NG:(eg + 1) * NG],
                        )

            emit_p3(0)
            r0 = emit_recip(0)
            emit_p4(0, r0)
            emit_p3(1)
            r1 = emit_recip(1)
            emit_p4(1, r1)
    nc.finalize()
    return nc


def build_in_maps(x, Wq, bq, Wk, bk, Wv, bv):
    x = np.asarray(x, dtype=np.float32)
    Wq = np.asarray(Wq, np.float32)
    Wk = np.asarray(Wk, np.float32)
    Wv = np.asarray(Wv, np.float32)
    bq = np.asarray(bq, np.float32)
    bv = np.asarray(bv, np.float32)

    M = (Wq.T @ Wk).astype(NP_MM)                      # [E, E]
    v2 = (Wk.T @ bq).astype(np.float32)                # [E]
    # mc[ech, p, ec*128+j] = M[ec*128+p, ech*128+j]  (ech-slab, 2KB DMA lines)
    mc = np.ascontiguousarray(
        M.reshape(EC, P, EC, P).transpose(2, 1, 0, 3).reshape(EC, P, E)
    )
    wvc = np.ascontiguousarray(Wv.T.astype(NP_MM).reshape(EC, P, E))
    bvb = np.broadcast_to(bv[None, :], (P, E)).copy()
    onesb = np.ones((P, 1), NP_MM)

    in_maps = []
    for c in range(8):
        b, h = divmod(c, 2)
        xb = x[b]
        if h == 1:
            xb = np.concatenate([xb[SQ:], xb[:SQ]], axis=0)
        xb16 = xb.astype(NP_MM)
        xtc = np.ascontiguousarray(xb16.T.reshape(EC, P, S))
        xsc = np.ascontiguousarray(xb16.reshape(SKC, P, E))
        t2 = (xb @ v2) * INV_SCALE                     # [S]
        t2b = np.ascontiguousarray(t2.reshape(SKC, P).T)  # [P, SKC]
        in_maps.append(
            dict(xtc=xtc, xsc=xsc, mc=mc, wvc=wvc, t2b=t2b, bvb=bvb,
                 onesb=onesb)
        )
    return in_maps


def kernel(x, Wq, bq, Wk, bk, Wv, bv):
    if "nc" not in _CACHE:
        _CACHE["nc"] = _build()
    nc = _CACHE["nc"]
    in_maps = build_in_maps(x, Wq, bq, Wk, bk, Wv, bv)
    res = run_bass_kernel_spmd(nc, in_maps, list(range(8)))

    out = np.empty((B, S, E), np.float32)
    for c in range(8):
        b, h = divmod(c, 2)
        out[b, h * SQ:(h + 1) * SQ, :] = res.results[c]["out"]
    return out
